# revision 57
# baseline (speedup 1.0000x reference)
"""Trainium2 Bass kernel for CachedGQA (32 q heads, 8 kv heads, head_dim 128, causal).

Sharding: tensor-parallel over kv heads -- core c owns kv head c and its 4 q heads.
Each core computes its q/k/v projections, causal GQA attention, and a partial
output through its 512-column slice of Wo (contraction-sharded); the host sums
the 8 partial outputs (the "all-reduce" of the row-sharded Wo).

Matmul precision strategy (fp8e4m3 DoubleRow, fp32 PSUM accumulation):
  The four projection GEMMs (Q/K/V and Wo) run in fp8 DoubleRow perf mode
  (2 fp8 weights per PE cell, two k-tiles contracted per pass) with full
  error compensation: every operand X is pre-scaled by a power of two and
  split into X = x1 + x2 with x1 = fp8(X), x2 = fp8(X - x1).  The product
  uses three DoubleRow terms -- x1*w1 (two k-tiles per pass), plus a combined
  correction pass (w2,w1)x(x1,x2) per k-tile -- dropping only the O(eps^2)
  x2*w2 term, so the result matches fp16 precision at 0.75x the fp16 PE
  cycle cost (DoubleRow contracts 256 elements per 0.5-cycle row).
  x and the weight splits are prepared on the host; the attention context is
  split on-device (DVE subtract) before the Wo GEMM.  Attention interior
  (scores, softmax, PV, denominator) stays fp16 for precision.

Device layout (from the fp16 baseline, unchanged where possible):
  - Host pre-transposes x -> xT [H, B*S] and all weight slices so every
    matmul contraction dim lands on SBUF partitions with no on-device
    transposes (except V, which uses 128x128 fp16 DMA-transposes).
  - Scores are computed transposed, sT[tk, tq] = (kT_chunk).T @ qT, so the
    probability tiles feed the PV matmul directly as the moving operand and no
    probability transposes are needed. Softmax skips the max-subtraction pass:
    a constant exp shift (exactly cancelling in softmax) keeps fp16 in range.
  - The softmax denominator comes from a constant [128,128] stationary
    matmul over the same probability tiles: every output partition receives
    the column sum, so the denominator arrives already broadcast across
    partitions (DVE lanes cannot read across partitions). The constant is
    1/SC so the DVE reciprocal directly yields SC/Z, folding the fp8 ctx
    pre-scale into the normalization multiply for free.
"""

import math
import os
import sys
from dataclasses import dataclass

import numpy as np
import ml_dtypes

if "/opt/trn_rl_repo" not in sys.path:
    sys.path.insert(0, "/opt/trn_rl_repo")

import concourse.bass as bass
import concourse.tile as tile
from concourse import bacc, mybir
from concourse import bass_utils

F8 = mybir.dt.float8e4
F16 = mybir.dt.float16
BF16 = mybir.dt.bfloat16
F32 = mybir.dt.float32
NP_F8 = ml_dtypes.float8_e4m3
DR = mybir.MatmulPerfMode.DoubleRow

P = 128


@dataclass(frozen=True)
class Cfg:
    B: int = 2      # batch
    S: int = 2048   # sequence length
    H: int = 4096   # hidden dim
    D: int = 128    # head dim (must be 128)
    G: int = 4      # q heads per core (one kv-head group)
    TT: int = 512   # token tile (free dim of most matmuls)

    @property
    def T(self):
        return self.B * self.S

    @property
    def M(self):
        return self.G * self.D  # per-core q/ctx features

    @property
    def HC(self):
        return self.H // P


FULL = Cfg()
N_CORES = 8
# Constant shift inside exp (cancels exactly in softmax). Chosen so the
# largest exp argument (~17.9 on this problem's data) stays ~3x under the
# fp16 max while keeping early-token probabilities out of fp16 subnormals.
EXP_SHIFT = -8.0
# fp8 pre-scales (powers of two; quantization error is scale-invariant, these
# just center each tensor's range inside e4m3's [2^-6, 240] normal band).
SX = 16.0     # x:   |x|max ~5.4  -> ~87
SW = 1024.0   # W:   |W|max ~0.11 -> ~111
SC = 8.0      # ctx: |ctx|<=|v|max ~6 -> ~48
QKV_DRAIN = 1.0 / (SX * SW)
WO_DRAIN = 1.0 / (SC * SW)


def emit_kernel(tc, cfg, xc, wq, wk, wv, wo, msk_d, out):
    nc = tc.nc
    B, S, H, D, G, TT = cfg.B, cfg.S, cfg.H, cfg.D, cfg.G, cfg.TT
    T, M, HC = cfg.T, cfg.M, cfg.HC
    assert D == P and TT % P == 0 and S % TT == 0 and H % 512 == 0
    scale = 1.0 / math.sqrt(D)
    PS_BUFS = {"s": 6, "o": 2}
    Exp = mybir.ActivationFunctionType.Exp
    Sub = mybir.AluOpType.subtract

    with (
        tc.tile_pool(name="persist", bufs=1) as persist,
        tc.tile_pool(name="psum_mm", bufs=3, space="PSUM") as psum_mm,
    ):
        qt = persist.tile([P, G, T], F16, name="qt")          # q^T per head [d, t]
        # Wo weights live in the persistent pool so their DMA can be issued
        # mid-phase-1 (DMA has slack there) and be resident before the first
        # Wo chunk right after batch-0's first attention tiles
        wo_s = persist.tile([P, G, 2, H], F8, name="wo_s")
        kt = persist.tile([P, T], F16, name="kt")             # k^T [d, t]
        vs = persist.tile([P, T // P, P], F16, name="vs")     # v [t-chunk, d]
        msk = persist.tile([P, 2 * TT - P], BF16, name="msk")  # causal staircase
        summat = persist.tile([P, P], BF16, name="summat")     # 1/SC everywhere
        expb = persist.tile([P, 1], F32, name="expb")  # exp bias (cancels in softmax)
        nc.sync.dma_start(msk, msk_d)
        nc.vector.memset(summat, 1.0 / SC)
        nc.vector.memset(expb, EXP_SHIFT)

        # ---------------- phase 1: q/k/v projections (fp8 DR, compensated) ----
        with (
            tc.tile_pool(name="wproj", bufs=1) as wpool,
            tc.tile_pool(name="xin", bufs=2) as xpool,
            tc.tile_pool(name="vtmp", bufs=2) as vpool,
        ):
            # weight slot layout: [., ., 0, .] = w2 (residual), [., ., 1, .] = w1
            wq_s = wpool.tile([P, HC, 2, M], F8, name="wq_s")
            wk_s = wpool.tile([P, HC, 2, D], F8, name="wk_s")
            wv_s = wpool.tile([P, HC, 2, D], F8, name="wv_s")
            wq_r = wq.rearrange("(hc p) two m -> p hc two m", p=P)
            xc_r = xc.rearrange("(hc p) two t -> p hc two t", p=P)
            xtile0 = xpool.tile([P, HC, 2, TT], F8, name="xtile", tag="xtile")
            # interleave eighth-loads of x and Wq so the first matmuls can
            # start after ~1 MiB of DMA instead of after all weight loads
            wk_r = wk.rearrange("(hc p) two m -> p hc two m", p=P)
            wv_r = wv.rearrange("(hc p) two m -> p hc two m", p=P)
            # interleave eighth-loads of x and Wq (main-term slots x-s0/wq-s1
            # first within each eighth) so the first matmuls can start after
            # ~1 MiB of DMA instead of after all weight loads
            for q8 in range(8):
                hs = slice(q8 * HC // 8, (q8 + 1) * HC // 8)
                nc.sync.dma_start(xtile0[:, hs, 0, :], xc_r[:, hs, 0, 0:TT])
                nc.sync.dma_start(wq_s[:, hs, 1, :], wq_r[:, hs, 1, :])
                nc.sync.dma_start(xtile0[:, hs, 1, :], xc_r[:, hs, 1, 0:TT])
                nc.sync.dma_start(wq_s[:, hs, 0, :], wq_r[:, hs, 0, :])
            for sl in (1, 0):
                nc.sync.dma_start(wk_s[:, :, sl, :], wk_r[:, :, sl, :])
                nc.sync.dma_start(wv_s[:, :, sl, :], wv_r[:, :, sl, :])

            def proj_main(ps, wsb, gsl, xtile):
                # main term: x1*w1, two hc-blocks contracted per DR pass
                for i in range(HC // 2):
                    nc.tensor.matmul(
                        ps,
                        lhsT=wsb[:, 2 * i : 2 * i + 2, 1, gsl],
                        rhs=xtile[:, 2 * i : 2 * i + 2, 0, :],
                        start=(i == 0),
                        stop=False,
                        perf_mode=DR,
                    )

            def proj_corr(ps, wsb, gsl, xtile):
                # correction: w2*x1 + w1*x2 per hc-block in one DR pass
                for hc in range(HC):
                    nc.tensor.matmul(
                        ps,
                        lhsT=wsb[:, hc, :, gsl],
                        rhs=xtile[:, hc, :, :],
                        start=False,
                        stop=(hc == HC - 1),
                        perf_mode=DR,
                    )

            def proj_psum(ps, wsb, gsl, xtile):
                proj_main(ps, wsb, gsl, xtile)
                proj_corr(ps, wsb, gsl, xtile)

            wo_r = wo.rearrange("(g p) two o -> p g two o", p=P)
            for it in range(T // TT):
                t0 = it * TT
                if it == 0:
                    xtile = xtile0
                else:
                    xtile = xpool.tile([P, HC, 2, TT], F8, name="xtile", tag="xtile")
                    for sl in range(2):
                        nc.sync.dma_start(xtile[:, :, sl, :], xc_r[:, :, sl, t0 : t0 + TT])
                if it == 3:
                    for sl in range(2):
                        nc.sync.dma_start(wo_s[:, :, sl, :], wo_r[:, :, sl, :])

                def q_group(g):
                    ps_q = psum_mm.tile([P, TT], F32, name="ps_q", tag="s", bufs=PS_BUFS["s"])
                    proj_psum(ps_q, wq_s, slice(g * D, (g + 1) * D), xtile)
                    nc.scalar.mul(qt[:, g, t0 : t0 + TT], ps_q, QKV_DRAIN)

                def q_groups_cold():
                    # tile 0: all mains first (need only the x-s0/wq-s1 DMA
                    # stream), corrections after -- matches DMA arrival and
                    # keeps the PE fed while residual slots stream in
                    pss = []
                    for g in range(G):
                        ps_q = psum_mm.tile([P, TT], F32, name="ps_q", tag="s", bufs=PS_BUFS["s"])
                        proj_main(ps_q, wq_s, slice(g * D, (g + 1) * D), xtile)
                        pss.append(ps_q)
                    for g in range(G):
                        proj_corr(pss[g], wq_s, slice(g * D, (g + 1) * D), xtile)
                        nc.scalar.mul(qt[:, g, t0 : t0 + TT], pss[g], QKV_DRAIN)

                def k_group():
                    ps_k = psum_mm.tile([P, TT], F32, name="ps_k", tag="s", bufs=PS_BUFS["s"])
                    proj_psum(ps_k, wk_s, slice(0, D), xtile)
                    nc.scalar.mul(kt[:, t0 : t0 + TT], ps_k, QKV_DRAIN)

                def v_group():
                    ps_v = psum_mm.tile([P, TT], F32, name="ps_v", tag="s", bufs=PS_BUFS["s"])
                    proj_psum(ps_v, wv_s, slice(0, D), xtile)
                    vt_tmp = vpool.tile([P, TT], F16, name="vt_tmp", tag="vt")
                    nc.scalar.mul(vt_tmp, ps_v, QKV_DRAIN)
                    for j in range(TT // P):
                        nc.sync.dma_start(
                            vs[:, t0 // P + j, :],
                            vt_tmp[:, j * P : (j + 1) * P],
                            transpose=True,
                        )

                for g in range(G):
                    q_group(g)
                k_group()
                v_group()

        # ---------------- phase 2: attention, phase 3: Wo ----------------
        with (
            tc.tile_pool(name="ph2", bufs=1) as ph2,
            tc.tile_pool(name="ptp", bufs=8) as ptp,
            tc.tile_pool(name="nrm", bufs=8) as nrm,
            tc.tile_pool(name="outp", bufs=4) as outp,
        ):
            # ctx slot layout: [., ., 0, .] = c1, [., ., 1, .] = c2 (residual)
            ctxc = ph2.tile([P, G, 2, T], F8, name="ctxc")

            def wo_group(tcn, io_, tag="s"):
                # one [128-token x 512-outcol] Wo psum group: 6 DR matmuls
                tsl = slice(tcn * P, (tcn + 1) * P)
                osl = slice(io_ * 512, (io_ + 1) * 512)
                ps_w = psum_mm.tile([P, 512], F32, name="ps_w", tag=tag, bufs=PS_BUFS[tag])
                # main: c1*w1, two g-blocks per DR pass
                for i in range(G // 2):
                    nc.tensor.matmul(
                        ps_w,
                        lhsT=ctxc[:, 2 * i : 2 * i + 2, 0, tsl],
                        rhs=wo_s[:, 2 * i : 2 * i + 2, 1, osl],
                        start=(i == 0),
                        stop=False,
                        perf_mode=DR,
                    )
                # correction: c1*w2 + c2*w1 per g-block
                for g in range(G):
                    nc.tensor.matmul(
                        ps_w,
                        lhsT=ctxc[:, g, :, tsl],
                        rhs=wo_s[:, g, :, osl],
                        start=False,
                        stop=(g == G - 1),
                        perf_mode=DR,
                    )
                ot = outp.tile([P, 512], F16, name="ot", tag="ot")
                # alternate copy engine: splits the psum-drain chain
                # across DVE and ACT so neither serializes the phase
                if (tcn + io_) % 2 == 0:
                    nc.vector.tensor_scalar_mul(ot, ps_w, WO_DRAIN)
                else:
                    nc.scalar.mul(ot, ps_w, WO_DRAIN)
                nc.sync.dma_start(out[tcn * P : (tcn + 1) * P, osl], ot)

            # pending Wo psum-groups, woven between attention chunks so the
            # PE always has independent matmuls while the exp ring drains
            pending_wo = []

            def weave_wo():
                if pending_wo:
                    tcn, io_ = pending_wo.pop(0)
                    wo_group(tcn, io_)

            LA = 3  # score-matmul lookahead (hides the ACT exp latency)

            def attention_tile(b, g, iq):
                        tq0 = iq * TT
                        nch = tq0 // P + TT // P  # causal: tk chunks <= tq tile end
                        ps_o = psum_mm.tile([P, TT], F32, name="ps_o", tag="o", bufs=PS_BUFS["o"])
                        # bf16 probability accumulator: bf16's exponent range
                        # removes any overflow bound, so ALL nch prob tiles
                        # sum into one tile on DVE (2-byte 2x mode), and the
                        # denominator becomes a single matmul per tile. The
                        # summat constant 1/SC folds the fp8 ctx pre-scale
                        # into the reciprocal; the matmul broadcast puts Z on
                        # every partition (DVE lanes cannot cross partitions).
                        acc = ptp.tile([P, TT], BF16, name="acc", tag="acc", bufs=2)
                        pts_info = [None] * nch

                        def stage_a(ic):  # score matmul + exp (+ mask)
                            tk0 = ic * P
                            o = tk0 - tq0
                            c0 = max(o, 0)  # cols below the causal boundary
                            ps_s = psum_mm.tile([P, TT], F32, name="ps_s", tag="s", bufs=PS_BUFS["s"])
                            nc.tensor.matmul(
                                ps_s[:, c0:],
                                lhsT=kt[:, b * S + tk0 : b * S + tk0 + P],
                                rhs=qt[:, g, b * S + tq0 + c0 : b * S + tq0 + TT],
                                start=True,
                                stop=True,
                            )
                            pt = ptp.tile([P, TT], BF16, name="pt", tag="pt")
                            nc.scalar.activation(
                                pt[:, c0:], ps_s[:, c0:], Exp, bias=expb, scale=scale
                            )
                            if o >= 0:  # partially-masked diagonal chunk
                                # on the (otherwise idle) GPSIMD engine: keeps
                                # the mask multiply off the DVE queue, which
                                # carries the prob-accumulate + ctx chains
                                nc.gpsimd.tensor_tensor(
                                    pt[:, c0:], pt[:, c0:],
                                    msk[:, TT - P : 2 * TT - P - o],
                                    mybir.AluOpType.mult,
                                )
                            pts_info[ic] = (pt, c0, o)

                        def stage_b(ic):  # PV matmul + bf16 prob accumulate
                            pt, c0, o = pts_info[ic]
                            tk0 = ic * P
                            nc.tensor.matmul(
                                ps_o[:, c0:],
                                lhsT=vs[:, (b * S + tk0) // P, :],
                                rhs=pt[:, c0:],
                                start=(ic == 0),
                                stop=(ic == nch - 1),
                            )
                            if ic == 0:  # first chunk always has c0 == 0
                                nc.vector.tensor_copy(acc, pt)
                            else:
                                nc.vector.tensor_tensor(
                                    acc[:, c0:], acc[:, c0:], pt[:, c0:],
                                    mybir.AluOpType.add,
                                )

                        for ic in range(nch + LA):
                            if ic < nch:
                                stage_a(ic)
                            if ic >= LA:
                                stage_b(ic - LA)
                            if ic % 2 == 0:
                                weave_wo()
                            if ic == 2 and epilogues:
                                epilogues.pop(0)()

                        def epilogue():
                            # single denominator matmul over the bf16 prob
                            # accumulator, then normalize + fp8 ctx split.
                            # Runs lagged, a few chunks into the NEXT
                            # attention tile, so the PE never waits for the
                            # DVE accumulate chain.
                            ps_d = psum_mm.tile([P, TT], F32, name="ps_d", tag="s", bufs=PS_BUFS["s"])
                            nc.tensor.matmul(
                                ps_d, lhsT=summat, rhs=acc, start=True, stop=True
                            )
                            tsl = slice(b * S + tq0, b * S + tq0 + TT)
                            rec = nrm.tile([P, TT], F32, name="rec", tag="rec")
                            nc.vector.reciprocal(rec, ps_d)  # = SC/Z
                            ts_ = nrm.tile([P, TT], F32, name="ts", tag="ts")
                            nc.vector.tensor_mul(ts_, ps_o, rec)  # = SC*ctx
                            nc.vector.tensor_copy(ctxc[:, g, 0, tsl], ts_)  # c1 (fp8 RNE)
                            nc.vector.tensor_tensor(
                                ctxc[:, g, 1, tsl], ts_, ctxc[:, g, 0, tsl], Sub
                            )  # c2 = SC*ctx - c1
                            if g == G - 1:
                                # all 4 heads' ctx for this token tile now
                                # emitted -> its Wo groups may be woven
                                pending_wo.extend(
                                    ((b * S + tq0) // P + j, io_)
                                    for j in range(TT // P)
                                    for io_ in range(H // 512)
                                )

                        epilogues.append(epilogue)

            # iq-major attention: after token-tile iq finishes for all 4
            # heads, its 4 Wo chunks are ready. Emit them one attention tile
            # LATER (lagged interleave) so the ctx split chain of tile iq has
            # a full tile of attention latency to drain before the Wo matmuls
            # need its fp8 pairs -- this spreads Wo across the whole phase
            # without stalling the PE at each iq boundary.
            epilogues = []
            for b in range(B):
                for iq in range(S // TT):
                    for g in range(G):
                        attention_tile(b, g, iq)
            while epilogues:
                epilogues.pop(0)()
            # drain the remaining groups (the last token tiles' Wo)
            for i, (tcn, io_) in enumerate(pending_wo):
                wo_group(tcn, io_, tag="s" if i % 2 == 0 else "o")


def build_program(cfg, num_devices=N_CORES):
    nc = bacc.Bacc("TRN2", debug=False, enable_asserts=False, num_devices=num_devices)
    xc = nc.dram_tensor("xc", [cfg.H, 2, cfg.T], F8, kind="ExternalInput").ap()
    wq = nc.dram_tensor("wq", [cfg.H, 2, cfg.M], F8, kind="ExternalInput").ap()
    wk = nc.dram_tensor("wk", [cfg.H, 2, cfg.D], F8, kind="ExternalInput").ap()
    wv = nc.dram_tensor("wv", [cfg.H, 2, cfg.D], F8, kind="ExternalInput").ap()
    wo = nc.dram_tensor("wo", [cfg.M, 2, cfg.H], F8, kind="ExternalInput").ap()
    msk = nc.dram_tensor("msk", [P, 2 * cfg.TT - P], BF16, kind="ExternalInput").ap()
    out = nc.dram_tensor("out", [cfg.T, cfg.H], F16, kind="ExternalOutput").ap()
    with tile.TileContext(nc) as tc:
        emit_kernel(tc, cfg, xc, wq, wk, wv, wo, msk, out)
    nc.compile()
    return nc


def make_mask(cfg):
    j = np.arange(2 * cfg.TT - P)[None, :]
    p = np.arange(P)[:, None]
    return (j >= p + (cfg.TT - P)).astype(ml_dtypes.bfloat16)


def split_fp8(a, s):
    """Pre-scale by s and split into (hi, residual) fp8e4m3 pair."""
    a = a.astype(np.float32) * np.float32(s)
    a1 = a.astype(NP_F8)
    a2 = (a - a1.astype(np.float32)).astype(NP_F8)
    return a1, a2


def w_pair(Wslice):
    """Weight DRAM layout [K, 2, M]: slot0 = residual w2, slot1 = main w1."""
    w1, w2 = split_fp8(Wslice, SW)
    return np.ascontiguousarray(np.stack([w2, w1], axis=1))


_CACHE = {}


def kernel(x, Wq, Wk, Wv, Wo, _trace=False):
    cfg = FULL
    x = np.asarray(x, dtype=np.float32)
    xt = x.reshape(cfg.T, cfg.H).T
    x1, x2 = split_fp8(xt, SX)
    xc = np.ascontiguousarray(np.stack([x1, x2], axis=1))  # [H, 2, T]
    msk = make_mask(cfg)
    M, D = cfg.M, cfg.D
    Wq, Wk, Wv, Wo = (np.asarray(w) for w in (Wq, Wk, Wv, Wo))
    in_maps = []
    for c in range(N_CORES):
        in_maps.append({
            "xc": xc,
            "msk": msk,
            "wq": w_pair(Wq[c * M : (c + 1) * M, :].T),
            "wk": w_pair(Wk[c * D : (c + 1) * D, :].T),
            "wv": w_pair(Wv[c * D : (c + 1) * D, :].T),
            "wo": w_pair(Wo[:, c * M : (c + 1) * M].T),
        })

    if "nc" not in _CACHE:
        _CACHE["nc"] = build_program(cfg)
    nc = _CACHE["nc"]

    try:
        res = bass_utils.run_bass_kernel_spmd(
            nc, in_maps, core_ids=list(range(N_CORES)), trace=_trace
        )
    except ModuleNotFoundError:
        # BASS_TRACE set but the axon NTFF hook module is unavailable in this
        # container -- retry with tracing force-disabled.
        os.environ["BASS_NEVER_TRACE"] = "1"
        res = bass_utils.run_bass_kernel_spmd(
            nc, in_maps, core_ids=list(range(N_CORES))
        )
    acc = np.zeros((cfg.T, cfg.H), np.float32)
    for r in res.results:
        acc += r["out"].astype(np.float32)
    out = acc.reshape(cfg.B, cfg.S, cfg.H)
    if _trace:
        return out, res
    return out


# revision 58
# speedup vs baseline: 1.0399x; 1.0399x over previous
"""Trainium2 Bass kernel for CachedGQA (32 q heads, 8 kv heads, head_dim 128, causal).

Sharding: tensor-parallel over kv heads -- core c owns kv head c and its 4 q heads.
Each core computes its q/k/v projections, causal GQA attention, and a partial
output through its 512-column slice of Wo (contraction-sharded); the host sums
the 8 partial outputs (the "all-reduce" of the row-sharded Wo).

Matmul precision strategy (fp8e4m3 DoubleRow, fp32 PSUM accumulation):
  The four projection GEMMs (Q/K/V and Wo) run in fp8 DoubleRow perf mode
  (2 fp8 weights per PE cell, two k-tiles contracted per pass) with full
  error compensation: every operand X is pre-scaled by a power of two and
  split into X = x1 + x2 with x1 = fp8(X), x2 = fp8(X - x1).  The product
  uses three DoubleRow terms -- x1*w1 (two k-tiles per pass), plus a combined
  correction pass (w2,w1)x(x1,x2) per k-tile -- dropping only the O(eps^2)
  x2*w2 term, so the result matches fp16 precision at 0.75x the fp16 PE
  cycle cost (DoubleRow contracts 256 elements per 0.5-cycle row).
  x and the weight splits are prepared on the host; the attention context is
  split on-device (DVE subtract) before the Wo GEMM.  Attention interior
  (scores, softmax, PV, denominator) stays fp16 for precision.

Device layout (from the fp16 baseline, unchanged where possible):
  - Host pre-transposes x -> xT [H, B*S] and all weight slices so every
    matmul contraction dim lands on SBUF partitions with no on-device
    transposes (except V, which uses 128x128 fp16 DMA-transposes).
  - Scores are computed transposed, sT[tk, tq] = (kT_chunk).T @ qT, so the
    probability tiles feed the PV matmul directly as the moving operand and no
    probability transposes are needed. Softmax skips the max-subtraction pass:
    a constant exp shift (exactly cancelling in softmax) keeps fp16 in range.
  - The softmax denominator comes from a constant [128,128] stationary
    matmul over the same probability tiles: every output partition receives
    the column sum, so the denominator arrives already broadcast across
    partitions (DVE lanes cannot read across partitions). The constant is
    1/SC so the DVE reciprocal directly yields SC/Z, folding the fp8 ctx
    pre-scale into the normalization multiply for free.
"""

import math
import os
import sys
from dataclasses import dataclass

import numpy as np
import ml_dtypes

if "/opt/trn_rl_repo" not in sys.path:
    sys.path.insert(0, "/opt/trn_rl_repo")

import concourse.bass as bass
import concourse.tile as tile
from concourse import bacc, mybir
from concourse import bass_utils

F8 = mybir.dt.float8e4
F16 = mybir.dt.float16
BF16 = mybir.dt.bfloat16
F32 = mybir.dt.float32
NP_F8 = ml_dtypes.float8_e4m3
DR = mybir.MatmulPerfMode.DoubleRow

P = 128


@dataclass(frozen=True)
class Cfg:
    B: int = 2      # batch
    S: int = 2048   # sequence length
    H: int = 4096   # hidden dim
    D: int = 128    # head dim (must be 128)
    G: int = 4      # q heads per core (one kv-head group)
    TT: int = 512   # token tile (free dim of most matmuls)

    @property
    def T(self):
        return self.B * self.S

    @property
    def M(self):
        return self.G * self.D  # per-core q/ctx features

    @property
    def HC(self):
        return self.H // P


FULL = Cfg()
N_CORES = 8
# Constant shift inside exp (cancels exactly in softmax). Chosen so the
# largest exp argument (~17.9 on this problem's data) stays ~3x under the
# fp16 max while keeping early-token probabilities out of fp16 subnormals.
EXP_SHIFT = -8.0
# fp8 pre-scales (powers of two; quantization error is scale-invariant, these
# just center each tensor's range inside e4m3's [2^-6, 240] normal band).
SX = 16.0     # x:   |x|max ~5.4  -> ~87
SW = 1024.0   # W:   |W|max ~0.11 -> ~111
SC = 8.0      # ctx: |ctx|<=|v|max ~6 -> ~48
QKV_DRAIN = 1.0 / (SX * SW)
WO_DRAIN = 1.0 / (SC * SW)


def emit_kernel(tc, cfg, xc, wq, wk, wv, wo, msk_d, out):
    nc = tc.nc
    B, S, H, D, G, TT = cfg.B, cfg.S, cfg.H, cfg.D, cfg.G, cfg.TT
    T, M, HC = cfg.T, cfg.M, cfg.HC
    assert D == P and TT % P == 0 and S % TT == 0 and H % 512 == 0
    scale = 1.0 / math.sqrt(D)
    PS_BUFS = {"s": 6, "o": 2}
    Exp = mybir.ActivationFunctionType.Exp
    Sub = mybir.AluOpType.subtract

    with (
        tc.tile_pool(name="persist", bufs=1) as persist,
        tc.tile_pool(name="psum_mm", bufs=3, space="PSUM") as psum_mm,
    ):
        qt = persist.tile([P, G, T], F16, name="qt")          # q^T per head [d, t]
        # Wo weights live in the persistent pool so their DMA can be issued
        # mid-phase-1 (DMA has slack there) and be resident before the first
        # Wo chunk right after batch-0's first attention tiles
        wo_s = persist.tile([P, G, 2, H], F8, name="wo_s")
        kt = persist.tile([P, T], F16, name="kt")             # k^T [d, t]
        vs = persist.tile([P, T // P, P], F16, name="vs")     # v [t-chunk, d]
        msk = persist.tile([P, 2 * TT - P], BF16, name="msk")  # causal staircase
        summat = persist.tile([P, P], BF16, name="summat")     # 1/SC everywhere
        expb = persist.tile([P, 1], F32, name="expb")  # exp bias (cancels in softmax)
        nc.sync.dma_start(msk, msk_d)
        nc.vector.memset(summat, 1.0 / SC)
        nc.vector.memset(expb, EXP_SHIFT)

        # ---------------- phase 1: q/k/v projections (fp8 DR, compensated) ----
        with (
            tc.tile_pool(name="wproj", bufs=1) as wpool,
            tc.tile_pool(name="xin", bufs=2) as xpool,
            tc.tile_pool(name="vtmp", bufs=2) as vpool,
        ):
            # weight slot layout: [., ., 0, .] = w2 (residual), [., ., 1, .] = w1
            wq_s = wpool.tile([P, HC, 2, M], F8, name="wq_s")
            wk_s = wpool.tile([P, HC, 2, D], F8, name="wk_s")
            wv_s = wpool.tile([P, HC, 2, D], F8, name="wv_s")
            wq_r = wq.rearrange("(hc p) two m -> p hc two m", p=P)
            xc_r = xc.rearrange("(hc p) two t -> p hc two t", p=P)
            xtile0 = xpool.tile([P, HC, 2, TT], F8, name="xtile", tag="xtile")
            # interleave eighth-loads of x and Wq so the first matmuls can
            # start after ~1 MiB of DMA instead of after all weight loads
            wk_r = wk.rearrange("(hc p) two m -> p hc two m", p=P)
            wv_r = wv.rearrange("(hc p) two m -> p hc two m", p=P)
            # interleave eighth-loads of x and Wq (main-term slots x-s0/wq-s1
            # first within each eighth) so the first matmuls can start after
            # ~1 MiB of DMA instead of after all weight loads
            for q8 in range(8):
                hs = slice(q8 * HC // 8, (q8 + 1) * HC // 8)
                nc.sync.dma_start(xtile0[:, hs, 0, :], xc_r[:, hs, 0, 0:TT])
                nc.sync.dma_start(wq_s[:, hs, 1, :], wq_r[:, hs, 1, :])
                nc.sync.dma_start(xtile0[:, hs, 1, :], xc_r[:, hs, 1, 0:TT])
                nc.sync.dma_start(wq_s[:, hs, 0, :], wq_r[:, hs, 0, :])
            for sl in (1, 0):
                nc.sync.dma_start(wk_s[:, :, sl, :], wk_r[:, :, sl, :])
                nc.sync.dma_start(wv_s[:, :, sl, :], wv_r[:, :, sl, :])

            def proj_main(ps, wsb, gsl, xtile):
                # main term: x1*w1, two hc-blocks contracted per DR pass
                for i in range(HC // 2):
                    nc.tensor.matmul(
                        ps,
                        lhsT=wsb[:, 2 * i : 2 * i + 2, 1, gsl],
                        rhs=xtile[:, 2 * i : 2 * i + 2, 0, :],
                        start=(i == 0),
                        stop=False,
                        perf_mode=DR,
                    )

            def proj_corr(ps, wsb, gsl, xtile):
                # correction: w2*x1 + w1*x2 per hc-block in one DR pass
                for hc in range(HC):
                    nc.tensor.matmul(
                        ps,
                        lhsT=wsb[:, hc, :, gsl],
                        rhs=xtile[:, hc, :, :],
                        start=False,
                        stop=(hc == HC - 1),
                        perf_mode=DR,
                    )

            def proj_psum(ps, wsb, gsl, xtile):
                proj_main(ps, wsb, gsl, xtile)
                proj_corr(ps, wsb, gsl, xtile)

            wo_r = wo.rearrange("(g p) two o -> p g two o", p=P)
            for it in range(T // TT):
                t0 = it * TT
                if it == 0:
                    xtile = xtile0
                else:
                    xtile = xpool.tile([P, HC, 2, TT], F8, name="xtile", tag="xtile")
                    for sl in range(2):
                        nc.sync.dma_start(xtile[:, :, sl, :], xc_r[:, :, sl, t0 : t0 + TT])
                if it == 3:
                    for sl in range(2):
                        nc.sync.dma_start(wo_s[:, :, sl, :], wo_r[:, :, sl, :])

                def q_group(g):
                    ps_q = psum_mm.tile([P, TT], F32, name="ps_q", tag="s", bufs=PS_BUFS["s"])
                    proj_psum(ps_q, wq_s, slice(g * D, (g + 1) * D), xtile)
                    nc.scalar.mul(qt[:, g, t0 : t0 + TT], ps_q, QKV_DRAIN)

                def q_groups_cold():
                    # tile 0: all mains first (need only the x-s0/wq-s1 DMA
                    # stream), corrections after -- matches DMA arrival and
                    # keeps the PE fed while residual slots stream in
                    pss = []
                    for g in range(G):
                        ps_q = psum_mm.tile([P, TT], F32, name="ps_q", tag="s", bufs=PS_BUFS["s"])
                        proj_main(ps_q, wq_s, slice(g * D, (g + 1) * D), xtile)
                        pss.append(ps_q)
                    for g in range(G):
                        proj_corr(pss[g], wq_s, slice(g * D, (g + 1) * D), xtile)
                        nc.scalar.mul(qt[:, g, t0 : t0 + TT], pss[g], QKV_DRAIN)

                def k_group():
                    ps_k = psum_mm.tile([P, TT], F32, name="ps_k", tag="s", bufs=PS_BUFS["s"])
                    proj_psum(ps_k, wk_s, slice(0, D), xtile)
                    nc.scalar.mul(kt[:, t0 : t0 + TT], ps_k, QKV_DRAIN)

                def v_group():
                    ps_v = psum_mm.tile([P, TT], F32, name="ps_v", tag="s", bufs=PS_BUFS["s"])
                    proj_psum(ps_v, wv_s, slice(0, D), xtile)
                    vt_tmp = vpool.tile([P, TT], F16, name="vt_tmp", tag="vt")
                    nc.scalar.mul(vt_tmp, ps_v, QKV_DRAIN)
                    for j in range(TT // P):
                        nc.sync.dma_start(
                            vs[:, t0 // P + j, :],
                            vt_tmp[:, j * P : (j + 1) * P],
                            transpose=True,
                        )

                for g in range(G):
                    q_group(g)
                k_group()
                v_group()

        # ---------------- phase 2: attention, phase 3: Wo ----------------
        with (
            tc.tile_pool(name="ph2", bufs=1) as ph2,
            tc.tile_pool(name="ptp", bufs=8) as ptp,
            tc.tile_pool(name="nrm", bufs=8) as nrm,
            tc.tile_pool(name="outp", bufs=4) as outp,
        ):
            # ctx slot layout: [., ., 0, .] = c1, [., ., 1, .] = c2 (residual)
            ctxc = ph2.tile([P, G, 2, T], F8, name="ctxc")

            def wo_group(tcn, io_, tag="s"):
                # one [128-token x 512-outcol] Wo psum group: 6 DR matmuls
                tsl = slice(tcn * P, (tcn + 1) * P)
                osl = slice(io_ * 512, (io_ + 1) * 512)
                ps_w = psum_mm.tile([P, 512], F32, name="ps_w", tag=tag, bufs=PS_BUFS[tag])
                # main: c1*w1, two g-blocks per DR pass
                for i in range(G // 2):
                    nc.tensor.matmul(
                        ps_w,
                        lhsT=ctxc[:, 2 * i : 2 * i + 2, 0, tsl],
                        rhs=wo_s[:, 2 * i : 2 * i + 2, 1, osl],
                        start=(i == 0),
                        stop=False,
                        perf_mode=DR,
                    )
                # correction: c1*w2 + c2*w1 per g-block
                for g in range(G):
                    nc.tensor.matmul(
                        ps_w,
                        lhsT=ctxc[:, g, :, tsl],
                        rhs=wo_s[:, g, :, osl],
                        start=False,
                        stop=(g == G - 1),
                        perf_mode=DR,
                    )
                ot = outp.tile([P, 512], F16, name="ot", tag="ot")
                # alternate copy engine: splits the psum-drain chain
                # across DVE and ACT so neither serializes the phase
                if (tcn + io_) % 2 == 0:
                    nc.vector.tensor_scalar_mul(ot, ps_w, WO_DRAIN)
                else:
                    nc.scalar.mul(ot, ps_w, WO_DRAIN)
                nc.sync.dma_start(out[tcn * P : (tcn + 1) * P, osl], ot)

            # pending Wo psum-groups, woven between attention chunks so the
            # PE always has independent matmuls while the exp ring drains
            pending_wo = []

            def weave_wo():
                if pending_wo:
                    tcn, io_ = pending_wo.pop(0)
                    wo_group(tcn, io_)

            LA = 3  # score-matmul lookahead (hides the ACT exp latency)

            def attention_tile(b, g, iq):
                        tq0 = iq * TT
                        nch = tq0 // P + TT // P  # causal: tk chunks <= tq tile end
                        ps_o = psum_mm.tile([P, TT], F32, name="ps_o", tag="o", bufs=PS_BUFS["o"])
                        # bf16 probability accumulator: bf16's exponent range
                        # removes any overflow bound, so ALL nch prob tiles
                        # sum into one tile on DVE (2-byte 2x mode), and the
                        # denominator becomes a single matmul per tile. The
                        # summat constant 1/SC folds the fp8 ctx pre-scale
                        # into the reciprocal; the matmul broadcast puts Z on
                        # every partition (DVE lanes cannot cross partitions).
                        acc = ptp.tile([P, TT], BF16, name="acc", tag="acc", bufs=2)
                        pts_info = [None] * nch

                        def stage_a(ic):  # score matmul + exp (+ mask)
                            tk0 = ic * P
                            o = tk0 - tq0
                            c0 = max(o, 0)  # cols below the causal boundary
                            ps_s = psum_mm.tile([P, TT], F32, name="ps_s", tag="s", bufs=PS_BUFS["s"])
                            nc.tensor.matmul(
                                ps_s[:, c0:],
                                lhsT=kt[:, b * S + tk0 : b * S + tk0 + P],
                                rhs=qt[:, g, b * S + tq0 + c0 : b * S + tq0 + TT],
                                start=True,
                                stop=True,
                            )
                            pt = ptp.tile([P, TT], BF16, name="pt", tag="pt")
                            nc.scalar.activation(
                                pt[:, c0:], ps_s[:, c0:], Exp, bias=expb, scale=scale
                            )
                            if o >= 0:  # partially-masked diagonal chunk
                                nc.vector.tensor_mul(
                                    pt[:, c0:], pt[:, c0:],
                                    msk[:, TT - P : 2 * TT - P - o],
                                )
                            pts_info[ic] = (pt, c0, o)

                        def stage_b(ic):  # PV matmul + bf16 prob accumulate
                            pt, c0, o = pts_info[ic]
                            tk0 = ic * P
                            nc.tensor.matmul(
                                ps_o[:, c0:],
                                lhsT=vs[:, (b * S + tk0) // P, :],
                                rhs=pt[:, c0:],
                                start=(ic == 0),
                                stop=(ic == nch - 1),
                            )
                            if ic == 0:  # first chunk always has c0 == 0
                                nc.vector.tensor_copy(acc, pt)
                            else:
                                nc.vector.tensor_tensor(
                                    acc[:, c0:], acc[:, c0:], pt[:, c0:],
                                    mybir.AluOpType.add,
                                )

                        for ic in range(nch + LA):
                            if ic < nch:
                                stage_a(ic)
                            if ic >= LA:
                                stage_b(ic - LA)
                            if ic % 2 == 0:
                                weave_wo()
                            if ic == 2 and epilogues:
                                epilogues.pop(0)()

                        def epilogue():
                            # single denominator matmul over the bf16 prob
                            # accumulator, then normalize + fp8 ctx split.
                            # Runs lagged, a few chunks into the NEXT
                            # attention tile, so the PE never waits for the
                            # DVE accumulate chain.
                            ps_d = psum_mm.tile([P, TT], F32, name="ps_d", tag="s", bufs=PS_BUFS["s"])
                            nc.tensor.matmul(
                                ps_d, lhsT=summat, rhs=acc, start=True, stop=True
                            )
                            tsl = slice(b * S + tq0, b * S + tq0 + TT)
                            rec = nrm.tile([P, TT], F32, name="rec", tag="rec")
                            nc.vector.reciprocal(rec, ps_d)  # = SC/Z
                            ts_ = nrm.tile([P, TT], F32, name="ts", tag="ts")
                            nc.vector.tensor_mul(ts_, ps_o, rec)  # = SC*ctx
                            nc.vector.tensor_copy(ctxc[:, g, 0, tsl], ts_)  # c1 (fp8 RNE)
                            nc.vector.tensor_tensor(
                                ctxc[:, g, 1, tsl], ts_, ctxc[:, g, 0, tsl], Sub
                            )  # c2 = SC*ctx - c1
                            if g == G - 1:
                                # all 4 heads' ctx for this token tile now
                                # emitted -> its Wo groups may be woven
                                pending_wo.extend(
                                    ((b * S + tq0) // P + j, io_)
                                    for j in range(TT // P)
                                    for io_ in range(H // 512)
                                )

                        epilogues.append(epilogue)

            # iq-major attention: after token-tile iq finishes for all 4
            # heads, its 4 Wo chunks are ready. Emit them one attention tile
            # LATER (lagged interleave) so the ctx split chain of tile iq has
            # a full tile of attention latency to drain before the Wo matmuls
            # need its fp8 pairs -- this spreads Wo across the whole phase
            # without stalling the PE at each iq boundary.
            epilogues = []
            for b in range(B):
                for iq in range(S // TT):
                    for g in range(G):
                        attention_tile(b, g, iq)
            while epilogues:
                epilogues.pop(0)()
            # drain the remaining groups (the last token tiles' Wo)
            for i, (tcn, io_) in enumerate(pending_wo):
                wo_group(tcn, io_, tag="s" if i % 2 == 0 else "o")


def build_program(cfg, num_devices=N_CORES):
    nc = bacc.Bacc("TRN2", debug=False, enable_asserts=False, num_devices=num_devices)
    xc = nc.dram_tensor("xc", [cfg.H, 2, cfg.T], F8, kind="ExternalInput").ap()
    wq = nc.dram_tensor("wq", [cfg.H, 2, cfg.M], F8, kind="ExternalInput").ap()
    wk = nc.dram_tensor("wk", [cfg.H, 2, cfg.D], F8, kind="ExternalInput").ap()
    wv = nc.dram_tensor("wv", [cfg.H, 2, cfg.D], F8, kind="ExternalInput").ap()
    wo = nc.dram_tensor("wo", [cfg.M, 2, cfg.H], F8, kind="ExternalInput").ap()
    msk = nc.dram_tensor("msk", [P, 2 * cfg.TT - P], BF16, kind="ExternalInput").ap()
    out = nc.dram_tensor("out", [cfg.T, cfg.H], F16, kind="ExternalOutput").ap()
    with tile.TileContext(nc) as tc:
        emit_kernel(tc, cfg, xc, wq, wk, wv, wo, msk, out)
    nc.compile()
    return nc


def make_mask(cfg):
    j = np.arange(2 * cfg.TT - P)[None, :]
    p = np.arange(P)[:, None]
    return (j >= p + (cfg.TT - P)).astype(ml_dtypes.bfloat16)


def split_fp8(a, s):
    """Pre-scale by s and split into (hi, residual) fp8e4m3 pair."""
    a = a.astype(np.float32) * np.float32(s)
    a1 = a.astype(NP_F8)
    a2 = (a - a1.astype(np.float32)).astype(NP_F8)
    return a1, a2


def w_pair(Wslice):
    """Weight DRAM layout [K, 2, M]: slot0 = residual w2, slot1 = main w1."""
    w1, w2 = split_fp8(Wslice, SW)
    return np.ascontiguousarray(np.stack([w2, w1], axis=1))


_CACHE = {}


def kernel(x, Wq, Wk, Wv, Wo, _trace=False):
    cfg = FULL
    x = np.asarray(x, dtype=np.float32)
    xt = x.reshape(cfg.T, cfg.H).T
    x1, x2 = split_fp8(xt, SX)
    xc = np.ascontiguousarray(np.stack([x1, x2], axis=1))  # [H, 2, T]
    msk = make_mask(cfg)
    M, D = cfg.M, cfg.D
    Wq, Wk, Wv, Wo = (np.asarray(w) for w in (Wq, Wk, Wv, Wo))
    in_maps = []
    for c in range(N_CORES):
        in_maps.append({
            "xc": xc,
            "msk": msk,
            "wq": w_pair(Wq[c * M : (c + 1) * M, :].T),
            "wk": w_pair(Wk[c * D : (c + 1) * D, :].T),
            "wv": w_pair(Wv[c * D : (c + 1) * D, :].T),
            "wo": w_pair(Wo[:, c * M : (c + 1) * M].T),
        })

    if "nc" not in _CACHE:
        _CACHE["nc"] = build_program(cfg)
    nc = _CACHE["nc"]

    try:
        res = bass_utils.run_bass_kernel_spmd(
            nc, in_maps, core_ids=list(range(N_CORES)), trace=_trace
        )
    except ModuleNotFoundError:
        # BASS_TRACE set but the axon NTFF hook module is unavailable in this
        # container -- retry with tracing force-disabled.
        os.environ["BASS_NEVER_TRACE"] = "1"
        res = bass_utils.run_bass_kernel_spmd(
            nc, in_maps, core_ids=list(range(N_CORES))
        )
    acc = np.zeros((cfg.T, cfg.H), np.float32)
    for r in res.results:
        acc += r["out"].astype(np.float32)
    out = acc.reshape(cfg.B, cfg.S, cfg.H)
    if _trace:
        return out, res
    return out
